# revision 6
# baseline (speedup 1.0000x reference)
"""BoxHead MLP (nn_BoxHead_60997125537856) — TRN2 Bass/Tile kernel.

reference:
    h  = relu(X @ W1 + b1)     X:[8192,12544]  W1:[12544,1024]
    h  = relu(h @ W2 + b2)     W2:[1024,1024]
    cl = h @ Wc + bc           Wc:[1024,4]
    bx = h @ Wr + br           Wr:[1024,12]
    return (cl, bx)

Strategy: data-parallel over the 8192 proposals -> 1024 rows per core on
8 NeuronCores; weights replicated. On-chip dataflow keeps FEATURES on
the partition axis and rows on the free axis, so layer outputs feed the
next layer's contraction with no transposes:

    h1T = relu(W1.T @ X_cT)     [1024 feat, 1024 rows]  (X_cT host-transposed)
    h2T = relu(W2.T @ h1T)      [1024, 1024]
    oT  = Wcr.T @ h2T + bcr     [16, 1024]   (Wcr = concat(Wc, Wr))

Matmuls run in bf16 (fp32 PSUM accumulation): 1 cycle/row on the PE vs
4 for fp32, and half the HBM traffic. Row dim is split in 512-halves
(PSUM bank = 512 fp32); feature dim in two 512-groups so the 8 PSUM
banks hold one group x both halves, letting each stationary-weight load
serve two matmuls. Per group, W1 feature-columns stream once and X_cT
streams once per group (2x total) — ~79 MB/core vs ~410 us of PE work:
PE-bound at the ridge.
"""

import sys

if "/opt/trn_rl_repo" not in sys.path:
    sys.path.insert(0, "/opt/trn_rl_repo")

import numpy as np
import ml_dtypes

import concourse.bass as bass  # noqa: F401  (engine types via bacc/tile)
import concourse.mybir as mybir
import concourse.tile as tile
from concourse import bacc
from concourse.bass_utils import run_bass_kernel_spmd

N_CORES = 8
N_TOTAL = 8192
D = 12544
H = 1024
CO = 16  # 4 class logits + 12 box deltas
ROWS = N_TOTAL // N_CORES  # 1024 rows per core
P = 128
HALF = 512
DKT = D // P  # 98 k-tiles for layer 1
HKT = H // P  # 8 k-tiles for layers 2/heads

MM_DT = mybir.dt.bfloat16
NP_DT = ml_dtypes.bfloat16

_CACHE = {}


def _build():
    nc = bacc.Bacc("TRN2", target_bir_lowering=False, debug=False, num_devices=N_CORES)

    xT_ap = nc.dram_tensor("xT", [D, ROWS], MM_DT, kind="ExternalInput").ap()
    w1_ap = nc.dram_tensor("w1", [D, H], MM_DT, kind="ExternalInput").ap()
    w2_ap = nc.dram_tensor("w2", [H, H], MM_DT, kind="ExternalInput").ap()
    wcr_ap = nc.dram_tensor("wcr", [H, CO], MM_DT, kind="ExternalInput").ap()
    b1_ap = nc.dram_tensor("b1v", [H], mybir.dt.float32, kind="ExternalInput").ap()
    b2_ap = nc.dram_tensor("b2v", [H], mybir.dt.float32, kind="ExternalInput").ap()
    bcr_ap = nc.dram_tensor("bcrv", [CO, 1], mybir.dt.float32, kind="ExternalInput").ap()
    out_ap = nc.dram_tensor("outT", [CO, ROWS], mybir.dt.float32, kind="ExternalOutput").ap()

    RELU = mybir.ActivationFunctionType.Relu
    IDENT = mybir.ActivationFunctionType.Identity

    STREAM_BUFS = 12
    PRIME = 12  # stream k-tiles DMA'd ahead of the weight-cache preloads

    with tile.TileContext(nc) as tc:
        with (
            tc.tile_pool(name="w1s", bufs=STREAM_BUFS) as w1s,
            tc.tile_pool(name="xs", bufs=STREAM_BUFS) as xs,
            tc.tile_pool(name="w2c", bufs=1) as w2c,
            tc.tile_pool(name="wcrc", bufs=1) as wcrc,
            tc.tile_pool(name="bias", bufs=1) as biasp,
            tc.tile_pool(name="scr", bufs=1) as scr,
            tc.tile_pool(name="h1c", bufs=1) as h1c,
            tc.tile_pool(name="h2c", bufs=1) as h2c,
            tc.tile_pool(name="outs", bufs=1) as outs,
            tc.tile_pool(name="ps", bufs=1, space="PSUM") as ps,
        ):
            # 8 persistent PSUM accumulators (one bank each)
            acc = [ps.tile([P, HALF], mybir.dt.float32, name=f"acc{j}") for j in range(8)]

            # --- PE warm-up: dummy matmuls on scratch while first DMAs land.
            # HAM releases the PE clock gate after ~3.4us of sustained busy;
            # doing the ramp on scratch data means real matmuls start at 2.4GHz.
            w_scr = scr.tile([P, P], MM_DT, name="w_scr")
            nc.vector.memset(w_scr[:], 0.0)
            x_scr = scr.tile([P, HALF], MM_DT, name="x_scr")
            nc.vector.memset(x_scr[:], 0.0)
            for j in range(14):
                nc.tensor.matmul(
                    acc[j % 2][:], lhsT=w_scr[:], rhs=x_scr[:], start=True, stop=True
                )

            # --- prime the layer-1 stream ahead of everything else ---
            def stream_pair(g, k):
                w1t = w1s.tile([P, 4 * P], MM_DT, name="w1t")
                nc.sync.dma_start(
                    out=w1t[:],
                    in_=w1_ap[k * P : (k + 1) * P, g * 4 * P : (g + 1) * 4 * P],
                )
                xt = xs.tile([P, ROWS], MM_DT, name="xt")
                nc.sync.dma_start(out=xt[:], in_=xT_ap[k * P : (k + 1) * P, :])
                return w1t, xt

            primed = [stream_pair(0, k) for k in range(PRIME)]

            h1 = [h1c.tile([P, ROWS], MM_DT, name=f"h1_{k}") for k in range(HKT)]
            h2 = [h2c.tile([P, ROWS], MM_DT, name=f"h2_{k}") for k in range(HKT)]

            # --- layer 1: h1T = relu(W1.T @ xT + b1) ---
            # feature groups g of 4 m-tiles; PSUM = 4 m-tiles x 2 row-halves.
            # Weight-cache preloads (w2/wcr/biases) are emitted between the
            # two groups so the DMA head serves only the layer-1 stream.
            b1t = b2t = bcrt = None
            w2t, wcrt = [], []
            for g in range(2):
                for k in range(DKT):
                    if g == 0 and k < PRIME:
                        w1t, xt = primed[k]
                    else:
                        w1t, xt = stream_pair(g, k)
                    st = k == 0
                    sp = k == DKT - 1
                    for m in range(4):
                        w_sl = w1t[:, m * P : (m + 1) * P]
                        nc.tensor.matmul(acc[2 * m][:], lhsT=w_sl, rhs=xt[:, :HALF], start=st, stop=sp)
                        nc.tensor.matmul(acc[2 * m + 1][:], lhsT=w_sl, rhs=xt[:, HALF:], start=st, stop=sp)

                if g == 0:
                    # constants / cached weights, needed from the first L1
                    # eviction (b1) and layer 2 (w2/wcr) onward
                    b1t = biasp.tile([P, HKT], mybir.dt.float32, name="b1t")
                    nc.sync.dma_start(out=b1t[:], in_=b1_ap.rearrange("(m p) -> p m", p=P))
                    b2t = biasp.tile([P, HKT], mybir.dt.float32, name="b2t")
                    nc.sync.dma_start(out=b2t[:], in_=b2_ap.rearrange("(m p) -> p m", p=P))
                    bcrt = biasp.tile([CO, 1], mybir.dt.float32, name="bcrt")
                    nc.sync.dma_start(out=bcrt[:], in_=bcr_ap[:])
                    for kk in range(HKT):
                        t = w2c.tile([P, H], MM_DT, name=f"w2_{kk}")
                        nc.sync.dma_start(out=t[:], in_=w2_ap[kk * P : (kk + 1) * P, :])
                        w2t.append(t)
                    for kk in range(HKT):
                        t = wcrc.tile([P, CO], MM_DT, name=f"wcr_{kk}")
                        nc.sync.dma_start(out=t[:], in_=wcr_ap[kk * P : (kk + 1) * P, :])
                        wcrt.append(t)

                for m in range(4):
                    mt = 4 * g + m
                    nc.scalar.activation(
                        h1[mt][:, :HALF], acc[2 * m][:], RELU, bias=b1t[:, mt : mt + 1]
                    )
                    nc.scalar.activation(
                        h1[mt][:, HALF:], acc[2 * m + 1][:], RELU, bias=b1t[:, mt : mt + 1]
                    )

            # --- layer 2: h2T = relu(W2.T @ h1T + b2) ---
            for g in range(2):
                for k in range(HKT):
                    st = k == 0
                    sp = k == HKT - 1
                    for m in range(4):
                        w_sl = w2t[k][:, (4 * g + m) * P : (4 * g + m + 1) * P]
                        nc.tensor.matmul(acc[2 * m][:], lhsT=w_sl, rhs=h1[k][:, :HALF], start=st, stop=sp)
                        nc.tensor.matmul(acc[2 * m + 1][:], lhsT=w_sl, rhs=h1[k][:, HALF:], start=st, stop=sp)
                for m in range(4):
                    mt = 4 * g + m
                    nc.scalar.activation(
                        h2[mt][:, :HALF], acc[2 * m][:], RELU, bias=b2t[:, mt : mt + 1]
                    )
                    nc.scalar.activation(
                        h2[mt][:, HALF:], acc[2 * m + 1][:], RELU, bias=b2t[:, mt : mt + 1]
                    )

            # --- heads: oT = Wcr.T @ h2T + bcr ---
            ot = outs.tile([CO, ROWS], mybir.dt.float32, name="ot")
            for h in range(2):
                pa = acc[h][:CO, :]
                for k in range(HKT):
                    nc.tensor.matmul(
                        pa,
                        lhsT=wcrt[k][:],
                        rhs=h2[k][:, h * HALF : (h + 1) * HALF],
                        start=(k == 0),
                        stop=(k == HKT - 1),
                    )
                nc.scalar.activation(
                    ot[:, h * HALF : (h + 1) * HALF], pa, IDENT, bias=bcrt[:, 0:1]
                )
            nc.sync.dma_start(out=out_ap[:], in_=ot[:])

    nc.compile()
    return nc


def kernel(feature_vectors, W1, b1, W2, b2, Wc, bc, Wr, br):
    if "nc" not in _CACHE:
        _CACHE["nc"] = _build()
    nc = _CACHE["nc"]

    X = np.asarray(feature_vectors, dtype=np.float32)
    w1 = np.asarray(W1, dtype=np.float32).astype(NP_DT)
    w2 = np.asarray(W2, dtype=np.float32).astype(NP_DT)
    wcr = np.concatenate(
        [np.asarray(Wc, np.float32), np.asarray(Wr, np.float32)], axis=1
    ).astype(NP_DT)
    b1v = np.asarray(b1, dtype=np.float32)
    b2v = np.asarray(b2, dtype=np.float32)
    bcrv = np.concatenate([np.asarray(bc, np.float32), np.asarray(br, np.float32)]).reshape(CO, 1)

    in_maps = []
    for c in range(N_CORES):
        xT = X[c * ROWS : (c + 1) * ROWS, :].T.astype(NP_DT)
        in_maps.append(
            {
                "xT": xT,
                "w1": w1,
                "w2": w2,
                "wcr": wcr,
                "b1v": b1v,
                "b2v": b2v,
                "bcrv": bcrv,
            }
        )

    res = run_bass_kernel_spmd(nc, in_maps, list(range(N_CORES)))
    _CACHE["last_results"] = res

    cl = np.empty((N_TOTAL, 4), np.float32)
    bx = np.empty((N_TOTAL, 12), np.float32)
    for c in range(N_CORES):
        oT = res.results[c]["outT"]
        cl[c * ROWS : (c + 1) * ROWS, :] = oT[:4].T
        bx[c * ROWS : (c + 1) * ROWS, :] = oT[4:].T
    return cl, bx


# revision 8
# speedup vs baseline: 1.0519x; 1.0519x over previous
"""BoxHead MLP (nn_BoxHead_60997125537856) — TRN2 Bass/Tile kernel.

reference:
    h  = relu(X @ W1 + b1)     X:[8192,12544]  W1:[12544,1024]
    h  = relu(h @ W2 + b2)     W2:[1024,1024]
    cl = h @ Wc + bc           Wc:[1024,4]
    bx = h @ Wr + br           Wr:[1024,12]
    return (cl, bx)

Strategy: data-parallel over the 8192 proposals -> 1024 rows per core on
8 NeuronCores; weights replicated. On-chip dataflow keeps FEATURES on
the partition axis and rows on the free axis, so layer outputs feed the
next layer's contraction with no transposes:

    h1T = relu(W1.T @ X_cT)     [1024 feat, 1024 rows]  (X_cT host-transposed)
    h2T = relu(W2.T @ h1T)      [1024, 1024]
    oT  = Wcr.T @ h2T + bcr     [16, 1024]   (Wcr = concat(Wc, Wr))

Matmuls run in bf16 (fp32 PSUM accumulation): 1 cycle/row on the PE vs
4 for fp32, and half the HBM traffic. Row dim is split in 512-halves
(PSUM bank = 512 fp32); feature dim in two 512-groups so the 8 PSUM
banks hold one group x both halves, letting each stationary-weight load
serve two matmuls. Per group, W1 feature-columns stream once and X_cT
streams once per group (2x total) — ~79 MB/core vs ~410 us of PE work:
PE-bound at the ridge.
"""

import sys

if "/opt/trn_rl_repo" not in sys.path:
    sys.path.insert(0, "/opt/trn_rl_repo")

import numpy as np
import ml_dtypes

import concourse.bass as bass  # noqa: F401  (engine types via bacc/tile)
import concourse.mybir as mybir
import concourse.tile as tile
from concourse import bacc
from concourse.bass_utils import run_bass_kernel_spmd

N_CORES = 8
N_TOTAL = 8192
D = 12544
H = 1024
CO = 16  # 4 class logits + 12 box deltas
ROWS = N_TOTAL // N_CORES  # 1024 rows per core
P = 128
HALF = 512
DKT = D // P  # 98 k-tiles for layer 1
HKT = H // P  # 8 k-tiles for layers 2/heads

MM_DT = mybir.dt.bfloat16
NP_DT = ml_dtypes.bfloat16

_CACHE = {}


def _build():
    nc = bacc.Bacc("TRN2", target_bir_lowering=False, debug=False, num_devices=N_CORES)

    xT_ap = nc.dram_tensor("xT", [D, ROWS], MM_DT, kind="ExternalInput").ap()
    w1_ap = nc.dram_tensor("w1", [D, H], MM_DT, kind="ExternalInput").ap()
    w2_ap = nc.dram_tensor("w2", [H, H], MM_DT, kind="ExternalInput").ap()
    wcr_ap = nc.dram_tensor("wcr", [H, CO], MM_DT, kind="ExternalInput").ap()
    b1_ap = nc.dram_tensor("b1v", [H], mybir.dt.float32, kind="ExternalInput").ap()
    b2_ap = nc.dram_tensor("b2v", [H], mybir.dt.float32, kind="ExternalInput").ap()
    bcr_ap = nc.dram_tensor("bcrv", [CO, 1], mybir.dt.float32, kind="ExternalInput").ap()
    out_ap = nc.dram_tensor("outT", [CO, ROWS], mybir.dt.float32, kind="ExternalOutput").ap()

    RELU = mybir.ActivationFunctionType.Relu
    IDENT = mybir.ActivationFunctionType.Identity

    STREAM_BUFS = 14
    PRIME = 14  # stream k-tiles DMA'd ahead of the weight-cache preloads

    with tile.TileContext(nc) as tc:
        with (
            tc.tile_pool(name="w1s", bufs=STREAM_BUFS) as w1s,
            tc.tile_pool(name="xs", bufs=STREAM_BUFS) as xs,
            tc.tile_pool(name="w2c", bufs=1) as w2c,
            tc.tile_pool(name="wcrc", bufs=1) as wcrc,
            tc.tile_pool(name="bias", bufs=1) as biasp,
            tc.tile_pool(name="scr", bufs=1) as scr,
            tc.tile_pool(name="h1c", bufs=1) as h1c,
            tc.tile_pool(name="h2c", bufs=1) as h2c,
            tc.tile_pool(name="outs", bufs=1) as outs,
            tc.tile_pool(name="ps", bufs=1, space="PSUM") as ps,
        ):
            # 8 persistent PSUM accumulators (one bank each)
            acc = [ps.tile([P, HALF], mybir.dt.float32, name=f"acc{j}") for j in range(8)]

            # --- PE warm-up: dummy matmuls on scratch while first DMAs land.
            # HAM releases the PE clock gate after ~3.4us of sustained busy;
            # doing the ramp on scratch data means real matmuls start at
            # 2.4GHz. One scratch tile + one memset keeps the dependency
            # chain minimal; the dummy results never leave PSUM (every real
            # accumulation below opens with start=True).
            scr_t = scr.tile([P, HALF], MM_DT, name="scr_t")
            nc.vector.memset(scr_t[:], 0.0)
            for j in range(14):
                nc.tensor.matmul(
                    acc[j % 2][:], lhsT=scr_t[:, :P], rhs=scr_t[:], start=True, stop=True
                )

            # --- prime the layer-1 stream ahead of everything else ---
            def stream_pair(g, k):
                w1t = w1s.tile([P, 4 * P], MM_DT, name="w1t")
                nc.sync.dma_start(
                    out=w1t[:],
                    in_=w1_ap[k * P : (k + 1) * P, g * 4 * P : (g + 1) * 4 * P],
                )
                xt = xs.tile([P, ROWS], MM_DT, name="xt")
                nc.sync.dma_start(out=xt[:], in_=xT_ap[k * P : (k + 1) * P, :])
                return w1t, xt

            primed = [stream_pair(0, k) for k in range(PRIME)]

            h1 = [h1c.tile([P, ROWS], MM_DT, name=f"h1_{k}") for k in range(HKT)]
            h2 = [h2c.tile([P, ROWS], MM_DT, name=f"h2_{k}") for k in range(HKT)]

            # --- layer 1: h1T = relu(W1.T @ xT + b1) ---
            # feature groups g of 4 m-tiles; PSUM = 4 m-tiles x 2 row-halves.
            # Weight-cache preloads (w2/wcr/biases) are emitted between the
            # two groups so the DMA head serves only the layer-1 stream.
            b1t = b2t = bcrt = None
            w2t, wcrt = [], []
            for g in range(2):
                for k in range(DKT):
                    if g == 0 and k < PRIME:
                        w1t, xt = primed[k]
                    else:
                        w1t, xt = stream_pair(g, k)
                    st = k == 0
                    sp = k == DKT - 1
                    for m in range(4):
                        w_sl = w1t[:, m * P : (m + 1) * P]
                        nc.tensor.matmul(acc[2 * m][:], lhsT=w_sl, rhs=xt[:, :HALF], start=st, stop=sp)
                        nc.tensor.matmul(acc[2 * m + 1][:], lhsT=w_sl, rhs=xt[:, HALF:], start=st, stop=sp)

                if g == 0:
                    # constants / cached weights, needed from the first L1
                    # eviction (b1) and layer 2 (w2/wcr) onward
                    b1t = biasp.tile([P, HKT], mybir.dt.float32, name="b1t")
                    nc.sync.dma_start(out=b1t[:], in_=b1_ap.rearrange("(m p) -> p m", p=P))
                    b2t = biasp.tile([P, HKT], mybir.dt.float32, name="b2t")
                    nc.sync.dma_start(out=b2t[:], in_=b2_ap.rearrange("(m p) -> p m", p=P))
                    bcrt = biasp.tile([CO, 1], mybir.dt.float32, name="bcrt")
                    nc.sync.dma_start(out=bcrt[:], in_=bcr_ap[:])
                    for kk in range(HKT):
                        t = w2c.tile([P, H], MM_DT, name=f"w2_{kk}")
                        nc.sync.dma_start(out=t[:], in_=w2_ap[kk * P : (kk + 1) * P, :])
                        w2t.append(t)
                    for kk in range(HKT):
                        t = wcrc.tile([P, CO], MM_DT, name=f"wcr_{kk}")
                        nc.sync.dma_start(out=t[:], in_=wcr_ap[kk * P : (kk + 1) * P, :])
                        wcrt.append(t)

                for m in range(4):
                    mt = 4 * g + m
                    nc.scalar.activation(
                        h1[mt][:, :HALF], acc[2 * m][:], RELU, bias=b1t[:, mt : mt + 1]
                    )
                    nc.scalar.activation(
                        h1[mt][:, HALF:], acc[2 * m + 1][:], RELU, bias=b1t[:, mt : mt + 1]
                    )

            # --- layer 2: h2T = relu(W2.T @ h1T + b2) ---
            for g in range(2):
                for k in range(HKT):
                    st = k == 0
                    sp = k == HKT - 1
                    for m in range(4):
                        w_sl = w2t[k][:, (4 * g + m) * P : (4 * g + m + 1) * P]
                        nc.tensor.matmul(acc[2 * m][:], lhsT=w_sl, rhs=h1[k][:, :HALF], start=st, stop=sp)
                        nc.tensor.matmul(acc[2 * m + 1][:], lhsT=w_sl, rhs=h1[k][:, HALF:], start=st, stop=sp)
                for m in range(4):
                    mt = 4 * g + m
                    nc.scalar.activation(
                        h2[mt][:, :HALF], acc[2 * m][:], RELU, bias=b2t[:, mt : mt + 1]
                    )
                    nc.scalar.activation(
                        h2[mt][:, HALF:], acc[2 * m + 1][:], RELU, bias=b2t[:, mt : mt + 1]
                    )

            # --- heads: oT = Wcr.T @ h2T + bcr ---
            ot = outs.tile([CO, ROWS], mybir.dt.float32, name="ot")
            for h in range(2):
                pa = acc[h][:CO, :]
                for k in range(HKT):
                    nc.tensor.matmul(
                        pa,
                        lhsT=wcrt[k][:],
                        rhs=h2[k][:, h * HALF : (h + 1) * HALF],
                        start=(k == 0),
                        stop=(k == HKT - 1),
                    )
                nc.scalar.activation(
                    ot[:, h * HALF : (h + 1) * HALF], pa, IDENT, bias=bcrt[:, 0:1]
                )
            nc.sync.dma_start(out=out_ap[:], in_=ot[:])

    nc.compile()
    return nc


def kernel(feature_vectors, W1, b1, W2, b2, Wc, bc, Wr, br):
    if "nc" not in _CACHE:
        _CACHE["nc"] = _build()
    nc = _CACHE["nc"]

    X = np.asarray(feature_vectors, dtype=np.float32)
    w1 = np.asarray(W1, dtype=np.float32).astype(NP_DT)
    w2 = np.asarray(W2, dtype=np.float32).astype(NP_DT)
    wcr = np.concatenate(
        [np.asarray(Wc, np.float32), np.asarray(Wr, np.float32)], axis=1
    ).astype(NP_DT)
    b1v = np.asarray(b1, dtype=np.float32)
    b2v = np.asarray(b2, dtype=np.float32)
    bcrv = np.concatenate([np.asarray(bc, np.float32), np.asarray(br, np.float32)]).reshape(CO, 1)

    in_maps = []
    for c in range(N_CORES):
        xT = X[c * ROWS : (c + 1) * ROWS, :].T.astype(NP_DT)
        in_maps.append(
            {
                "xT": xT,
                "w1": w1,
                "w2": w2,
                "wcr": wcr,
                "b1v": b1v,
                "b2v": b2v,
                "bcrv": bcrv,
            }
        )

    res = run_bass_kernel_spmd(nc, in_maps, list(range(N_CORES)))
    _CACHE["last_results"] = res

    cl = np.empty((N_TOTAL, 4), np.float32)
    bx = np.empty((N_TOTAL, 12), np.float32)
    for c in range(N_CORES):
        oT = res.results[c]["outT"]
        cl[c * ROWS : (c + 1) * ROWS, :] = oT[:4].T
        bx[c * ROWS : (c + 1) * ROWS, :] = oT[4:].T
    return cl, bx


# revision 9
# speedup vs baseline: 1.1059x; 1.0514x over previous
"""BoxHead MLP (nn_BoxHead_60997125537856) — TRN2 Bass/Tile kernel.

reference:
    h  = relu(X @ W1 + b1)     X:[8192,12544]  W1:[12544,1024]
    h  = relu(h @ W2 + b2)     W2:[1024,1024]
    cl = h @ Wc + bc           Wc:[1024,4]
    bx = h @ Wr + br           Wr:[1024,12]
    return (cl, bx)

Strategy: data-parallel over the 8192 proposals -> 1024 rows per core on
8 NeuronCores; weights replicated. On-chip dataflow keeps FEATURES on
the partition axis and rows on the free axis, so layer outputs feed the
next layer's contraction with no transposes:

    h1T = relu(W1.T @ X_cT)     [1024 feat, 1024 rows]  (X_cT host-transposed)
    h2T = relu(W2.T @ h1T)      [1024, 1024]
    oT  = Wcr.T @ h2T + bcr     [16, 1024]   (Wcr = concat(Wc, Wr))

Matmuls run in bf16 (fp32 PSUM accumulation): 1 cycle/row on the PE vs
4 for fp32, and half the HBM traffic. Row dim is split in 512-halves
(PSUM bank = 512 fp32); feature dim in two 512-groups so the 8 PSUM
banks hold one group x both halves, letting each stationary-weight load
serve two matmuls. Per group, W1 feature-columns stream once and X_cT
streams once per group (2x total) — ~79 MB/core vs ~410 us of PE work:
PE-bound at the ridge.
"""

import sys

if "/opt/trn_rl_repo" not in sys.path:
    sys.path.insert(0, "/opt/trn_rl_repo")

import numpy as np
import ml_dtypes

import concourse.bass as bass  # noqa: F401  (engine types via bacc/tile)
import concourse.mybir as mybir
import concourse.tile as tile
from concourse import bacc
from concourse.bass_utils import run_bass_kernel_spmd

N_CORES = 8
N_TOTAL = 8192
D = 12544
H = 1024
CO = 16  # 4 class logits + 12 box deltas
ROWS = N_TOTAL // N_CORES  # 1024 rows per core
P = 128
HALF = 512
DKT = D // P  # 98 k-tiles for layer 1
HKT = H // P  # 8 k-tiles for layers 2/heads

MM_DT = mybir.dt.bfloat16
NP_DT = ml_dtypes.bfloat16

_CACHE = {}


def _build():
    nc = bacc.Bacc("TRN2", target_bir_lowering=False, debug=False, num_devices=N_CORES)

    xT_ap = nc.dram_tensor("xT", [D, ROWS], MM_DT, kind="ExternalInput").ap()
    w1_ap = nc.dram_tensor("w1", [D, H], MM_DT, kind="ExternalInput").ap()
    w2_ap = nc.dram_tensor("w2", [H, H], MM_DT, kind="ExternalInput").ap()
    wcr_ap = nc.dram_tensor("wcr", [H, CO], MM_DT, kind="ExternalInput").ap()
    b1_ap = nc.dram_tensor("b1v", [H], mybir.dt.float32, kind="ExternalInput").ap()
    b2_ap = nc.dram_tensor("b2v", [H], mybir.dt.float32, kind="ExternalInput").ap()
    bcr_ap = nc.dram_tensor("bcrv", [CO, 1], mybir.dt.float32, kind="ExternalInput").ap()
    out_ap = nc.dram_tensor("outT", [CO, ROWS], mybir.dt.float32, kind="ExternalOutput").ap()

    RELU = mybir.ActivationFunctionType.Relu
    IDENT = mybir.ActivationFunctionType.Identity

    STREAM_BUFS = 16
    PRIME = 16  # stream k-tiles DMA'd ahead of the weight-cache preloads

    with tile.TileContext(nc) as tc:
        with (
            tc.tile_pool(name="w1s", bufs=STREAM_BUFS) as w1s,
            tc.tile_pool(name="xs", bufs=STREAM_BUFS) as xs,
            tc.tile_pool(name="w2c", bufs=1) as w2c,
            tc.tile_pool(name="wcrc", bufs=1) as wcrc,
            tc.tile_pool(name="bias", bufs=1) as biasp,
            tc.tile_pool(name="h1c", bufs=1) as h1c,
            tc.tile_pool(name="h2c", bufs=1) as h2c,
            tc.tile_pool(name="outs", bufs=1) as outs,
            tc.tile_pool(name="ps", bufs=1, space="PSUM") as ps,
        ):
            # 8 persistent PSUM accumulators (one bank each)
            acc = [ps.tile([P, HALF], mybir.dt.float32, name=f"acc{j}") for j in range(8)]


            def stream_pair(g, k):
                w1t = w1s.tile([P, 4 * P], MM_DT, name="w1t")
                nc.sync.dma_start(
                    out=w1t[:],
                    in_=w1_ap[k * P : (k + 1) * P, g * 4 * P : (g + 1) * 4 * P],
                )
                xt = xs.tile([P, ROWS], MM_DT, name="xt")
                nc.sync.dma_start(out=xt[:], in_=xT_ap[k * P : (k + 1) * P, :])
                return w1t, xt

            h1 = [h1c.tile([P, ROWS], MM_DT, name=f"h1_{k}") for k in range(HKT)]
            h2 = [h2c.tile([P, ROWS], MM_DT, name=f"h2_{k}") for k in range(HKT)]

            # --- PE warm-up: dummy matmuls while the first DMAs land. HAM
            # releases the PE clock gate after ~3.4us of sustained busy, so
            # the real stream starts at 2.4GHz. Reading h1[0] before its
            # first write needs no memset (values are irrelevant): the
            # warm-up has no cross-engine dependency and starts at PE boot.
            # Results never leave PSUM — the real accumulations below open
            # with start=True.
            for j in range(10):
                nc.tensor.matmul(
                    acc[j % 2][:], lhsT=h1[0][:, :P], rhs=h1[0][:, :HALF],
                    start=True, stop=True,
                )

            # --- prime the layer-1 stream ahead of everything else ---
            primed = [stream_pair(0, k) for k in range(PRIME)]

            # --- layer 1: h1T = relu(W1.T @ xT + b1) ---
            # feature groups g of 4 m-tiles; PSUM = 4 m-tiles x 2 row-halves.
            # Weight-cache preloads (w2/wcr/biases) are emitted between the
            # two groups so the DMA head serves only the layer-1 stream.
            b1t = b2t = bcrt = None
            w2t, wcrt = [], []
            for g in range(2):
                for k in range(DKT):
                    if g == 0 and k < PRIME:
                        w1t, xt = primed[k]
                    else:
                        w1t, xt = stream_pair(g, k)
                    st = k == 0
                    sp = k == DKT - 1
                    for m in range(4):
                        w_sl = w1t[:, m * P : (m + 1) * P]
                        nc.tensor.matmul(acc[2 * m][:], lhsT=w_sl, rhs=xt[:, :HALF], start=st, stop=sp)
                        nc.tensor.matmul(acc[2 * m + 1][:], lhsT=w_sl, rhs=xt[:, HALF:], start=st, stop=sp)
                    if g == 1 and k % 8 == 4 and k // 8 < HKT:
                        # stagger the 2MB w2 cache across the g=1 stream so
                        # it never bursts against the layer-1 loads
                        kk = k // 8
                        t = w2c.tile([P, H], MM_DT, name=f"w2_{kk}")
                        nc.sync.dma_start(out=t[:], in_=w2_ap[kk * P : (kk + 1) * P, :])
                        w2t.append(t)

                if g == 0:
                    # small constants, needed from the first L1 eviction (b1)
                    # and the heads onward (wcr) — ~40KB total, no burst risk
                    b1t = biasp.tile([P, HKT], mybir.dt.float32, name="b1t")
                    nc.sync.dma_start(out=b1t[:], in_=b1_ap.rearrange("(m p) -> p m", p=P))
                    b2t = biasp.tile([P, HKT], mybir.dt.float32, name="b2t")
                    nc.sync.dma_start(out=b2t[:], in_=b2_ap.rearrange("(m p) -> p m", p=P))
                    bcrt = biasp.tile([CO, 1], mybir.dt.float32, name="bcrt")
                    nc.sync.dma_start(out=bcrt[:], in_=bcr_ap[:])
                    for kk in range(HKT):
                        t = wcrc.tile([P, CO], MM_DT, name=f"wcr_{kk}")
                        nc.sync.dma_start(out=t[:], in_=wcr_ap[kk * P : (kk + 1) * P, :])
                        wcrt.append(t)

                for m in range(4):
                    mt = 4 * g + m
                    nc.scalar.activation(
                        h1[mt][:, :HALF], acc[2 * m][:], RELU, bias=b1t[:, mt : mt + 1]
                    )
                    nc.scalar.activation(
                        h1[mt][:, HALF:], acc[2 * m + 1][:], RELU, bias=b1t[:, mt : mt + 1]
                    )

            # --- layer 2: h2T = relu(W2.T @ h1T + b2) ---
            for g in range(2):
                for k in range(HKT):
                    st = k == 0
                    sp = k == HKT - 1
                    for m in range(4):
                        w_sl = w2t[k][:, (4 * g + m) * P : (4 * g + m + 1) * P]
                        nc.tensor.matmul(acc[2 * m][:], lhsT=w_sl, rhs=h1[k][:, :HALF], start=st, stop=sp)
                        nc.tensor.matmul(acc[2 * m + 1][:], lhsT=w_sl, rhs=h1[k][:, HALF:], start=st, stop=sp)
                for m in range(4):
                    mt = 4 * g + m
                    nc.scalar.activation(
                        h2[mt][:, :HALF], acc[2 * m][:], RELU, bias=b2t[:, mt : mt + 1]
                    )
                    nc.scalar.activation(
                        h2[mt][:, HALF:], acc[2 * m + 1][:], RELU, bias=b2t[:, mt : mt + 1]
                    )

            # --- heads: oT = Wcr.T @ h2T + bcr ---
            ot = outs.tile([CO, ROWS], mybir.dt.float32, name="ot")
            for h in range(2):
                pa = acc[h][:CO, :]
                for k in range(HKT):
                    nc.tensor.matmul(
                        pa,
                        lhsT=wcrt[k][:],
                        rhs=h2[k][:, h * HALF : (h + 1) * HALF],
                        start=(k == 0),
                        stop=(k == HKT - 1),
                    )
                nc.scalar.activation(
                    ot[:, h * HALF : (h + 1) * HALF], pa, IDENT, bias=bcrt[:, 0:1]
                )
            nc.sync.dma_start(out=out_ap[:], in_=ot[:])

    nc.compile()
    return nc


def kernel(feature_vectors, W1, b1, W2, b2, Wc, bc, Wr, br):
    if "nc" not in _CACHE:
        _CACHE["nc"] = _build()
    nc = _CACHE["nc"]

    X = np.asarray(feature_vectors, dtype=np.float32)
    w1 = np.asarray(W1, dtype=np.float32).astype(NP_DT)
    w2 = np.asarray(W2, dtype=np.float32).astype(NP_DT)
    wcr = np.concatenate(
        [np.asarray(Wc, np.float32), np.asarray(Wr, np.float32)], axis=1
    ).astype(NP_DT)
    b1v = np.asarray(b1, dtype=np.float32)
    b2v = np.asarray(b2, dtype=np.float32)
    bcrv = np.concatenate([np.asarray(bc, np.float32), np.asarray(br, np.float32)]).reshape(CO, 1)

    in_maps = []
    for c in range(N_CORES):
        xT = X[c * ROWS : (c + 1) * ROWS, :].T.astype(NP_DT)
        in_maps.append(
            {
                "xT": xT,
                "w1": w1,
                "w2": w2,
                "wcr": wcr,
                "b1v": b1v,
                "b2v": b2v,
                "bcrv": bcrv,
            }
        )

    res = run_bass_kernel_spmd(nc, in_maps, list(range(N_CORES)))
    _CACHE["last_results"] = res

    cl = np.empty((N_TOTAL, 4), np.float32)
    bx = np.empty((N_TOTAL, 12), np.float32)
    for c in range(N_CORES):
        oT = res.results[c]["outT"]
        cl[c * ROWS : (c + 1) * ROWS, :] = oT[:4].T
        bx[c * ROWS : (c + 1) * ROWS, :] = oT[4:].T
    return cl, bx
